# revision 24
# baseline (speedup 1.0000x reference)
"""AdaptiveEmbedding (adaptive-softmax style embedding lookup) on 8 TRN2
NeuronCores.

Sharding: data-parallel over tokens (4096 tokens/core); bucket tables and
projections are replicated in device DRAM as bf16. Within each core's shard,
tokens are sub-sharded host-side by (bucket, int16-addressable row-range
chunk) -- dma_gather/dma_scatter_add take int16 indices, so each gather unit
covers at most 32768 table rows.

Device program per core (one SPMD NEFF on cores 0-7):
  - bucket 0 (d=1024): dma_gather of hit rows -> DVE scale by sqrt(1024)=32
  - buckets 1-3 (d=256/64/16): transposed dma_gather produces e^T tiles
    ([d on partitions] x [tokens on free dim]) directly; TensorE matmuls
    against the replicated projection (PSUM f32 accumulate); scale-by-32
    PSUM->SBUF copy alternating between DVE and ScalarE.
  - output, two modes:
      DEVICE_SCATTER=True:  dma_scatter_add places each row at its token
        position across KOUT round-robin output tensors (avoids WAW
        serialization between scatter DMAs); host sums the disjoint partials.
      DEVICE_SCATTER=False (default, ~40us faster): each unit's rows are
        written contiguously (plain HWDGE DMA); the host unshard step places
        each sub-shard's rows at their token positions. This removes the
        per-row scatter descriptor generation, which is GpSimd-bound
        (~10ns/row on the Q7 SWDGE ucode) and is the kernel's bottleneck.

Output is bf16 on device (rel err ~3e-3), upcast to f32 on host.
"""
import math
import numpy as np
import ml_dtypes

N_VOCAB = 267735
STARTS = [0, 20000, 40000, 200000]
ENDS = [20000, 40000, 200000, N_VOCAB]
N_EMBEDS = [1024, 256, 64, 16]
N_CORES = 8
NEMB = 1024
SCALE = 32.0  # sqrt(1024)
KOUT = 4   # independent scatter target tensors
DEVICE_SCATTER = False  # False: device writes bucket-contiguous, host places rows
CHUNK = 32768  # int16-addressable rows per gather chunk
P = 128

# static unit list: (bucket, chunk_index)
UNITS = []
for _b in range(4):
    _nr = ENDS[_b] - STARTS[_b]
    for _c in range(math.ceil(_nr / CHUNK)):
        UNITS.append((_b, _c))


def _wrap16(a):
    # [N] -> [16, N/16] wrapped, replicated to 128 partitions
    w = a.reshape(-1, 16).T.astype(np.int16)
    return np.tile(w, (8, 1))


def _prep_host(inputs):
    x = np.asarray(inputs["x"]).astype(np.int64).reshape(-1)
    ntok = x.size
    assert ntok % N_CORES == 0, f"token count {ntok} not divisible by {N_CORES}"
    per = ntok // N_CORES

    bf = ml_dtypes.bfloat16
    tabs = {}
    for b in range(4):
        t = np.asarray(inputs[f"table{b}"], np.float32)
        d = N_EMBEDS[b]
        if d < P:  # pad rows to 128 elements (256B in bf16) for dma_gather
            tp = np.zeros((t.shape[0], P), np.float32)
            tp[:, :d] = t
            t = tp
        tb = t.astype(bf)
        nr = t.shape[0]
        for c in range(math.ceil(nr / CHUNK)):
            tabs[(b, c)] = np.ascontiguousarray(tb[c * CHUNK : (c + 1) * CHUNK])
    projs = {
        i: np.asarray(inputs[f"proj{i}"], np.float32).astype(bf) for i in (1, 2, 3)
    }

    # per-core, per-unit token lists. Tokens of each unit are dealt
    # round-robin across cores so per-core unit counts differ by <=1 --
    # this minimizes the max-over-cores capacity (= gather descriptor work).
    counts = {u: [] for u in UNITS}
    lists = [dict() for _ in range(N_CORES)]
    for (b, c) in UNITS:
        lo = STARTS[b] + c * CHUNK
        hi = min(STARTS[b] + (c + 1) * CHUNK, ENDS[b])
        g = np.nonzero((x >= lo) & (x < hi))[0]  # global positions
        for core in range(N_CORES):
            sel = g[core::N_CORES]
            lists[core][(b, c)] = (x[sel] - lo, sel)
            counts[(b, c)].append(len(sel))
    # core-local ordering (rank within the core's token subset) for the
    # int16 scatter positions of DEVICE_SCATTER mode
    orders = []
    for core in range(N_CORES):
        allg = np.concatenate([lists[core][u][1] for u in UNITS])
        orders.append(np.sort(allg))

    caps = {
        u: max(P, -(-max(counts[u]) // P) * P) for u in UNITS
    }  # multiple of 128, >= max count (gather/transposed-gather size)
    caps16 = {
        u: min(caps[u], max(16, -(-max(counts[u]) // 16) * 16)) for u in UNITS
    }  # multiple of 16 -- used for scatters and the b0 gather

    # meta tensor: per unit [idx_wrapped | pos_wrapped], concat along free dim
    metas = []
    meta_off = {}
    off = 0
    for u in UNITS:
        w = caps[u] // 16
        meta_off[u] = off
        off += 2 * w
    for core in range(N_CORES):
        cols = []
        for u in UNITS:
            lid, pos = lists[core][u]
            cap = caps[u]
            # pad with valid entries so every core has exactly `cap` live
            # indices: idx 0 (gathers row 0, discarded) and dummy output rows
            # (sliced off on the host).
            il = np.zeros(cap, np.int64)
            pl = per + P + (np.arange(cap, dtype=np.int64) % P)
            il[: len(lid)] = lid
            pl[: len(pos)] = np.searchsorted(orders[core], pos)
            cols.append(_wrap16(il))
            cols.append(_wrap16(pl))
        metas.append(np.concatenate(cols, axis=1))
    return per, tabs, projs, metas, caps, caps16, meta_off, lists, orders


def _build(per, tabs, projs, caps, caps16, meta_off, meta_w):
    import concourse.bass as bass
    import concourse.tile as tile
    from concourse import bacc, mybir

    bf = mybir.dt.bfloat16
    nc = bacc.Bacc("TRN2", target_bir_lowering=False, debug=False)

    tab_d = {
        u: nc.dram_tensor(f"tab{u[0]}_{u[1]}", list(tabs[u].shape), bf,
                          kind="ExternalInput")
        for u in UNITS
    }
    proj_d = {
        i: nc.dram_tensor(f"proj{i}", list(projs[i].shape), bf,
                          kind="ExternalInput")
        for i in (1, 2, 3)
    }
    meta_d = nc.dram_tensor("meta", [P, meta_w], mybir.dt.int16,
                            kind="ExternalInput")
    # KOUT independent output tensors (scatters round-robin over them) so
    # consecutive scatter DMAs have no WAW edge and overlap; rows are
    # disjoint across units, host sums the partials.
    if DEVICE_SCATTER:
        out_ds = [
            nc.dram_tensor(f"out{k}", [per + 2 * P, NEMB], bf, kind="ExternalOutput")
            for k in range(KOUT)
        ]
        outc = None
    else:
        ncap = sum(caps.values())
        outc = nc.dram_tensor("outc", [ncap, NEMB], bf, kind="ExternalOutput")
        out_ds = [outc]  # unused in this mode (warmup scatter is skipped)
        coff = {}
        off = 0
        for u in UNITS:
            coff[u] = off
            off += caps[u]

    with tile.TileContext(nc) as tc:
        with (
            tc.tile_pool(name="sb", bufs=1) as sb,
            tc.tile_pool(name="ps", bufs=4, space="PSUM") as ps,
        ):
            meta_t = sb.tile([P, meta_w], mybir.dt.int16, tag="meta")
            nc.gpsimd.dma_start(meta_t[:], meta_d.ap())

            # Warmup: tiny gather (+scatter in scatter mode) with no data
            # deps, so the one-time custom-op ucode load overlaps the
            # pipeline head.
            widx = sb.tile([P, 1], mybir.dt.int16, tag="widx")
            nc.gpsimd.memset(widx[:], 0)
            wpos = sb.tile([P, 1], mybir.dt.int16, tag="wpos")
            nc.gpsimd.memset(wpos[:], per + P)  # dummy output row
            wout = sb.tile([P, NEMB], bf, tag="wout")
            nc.gpsimd.dma_gather(
                out_ap=wout[:].rearrange("p (g e) -> p g e", e=NEMB),
                in_ap=proj_d[3].ap(),
                idxs_ap=widx[:],
                num_idxs=16,
                num_idxs_reg=16,
                elem_size=NEMB,
            )
            if DEVICE_SCATTER:
                nc.gpsimd.dma_scatter_add(
                    out_ap=out_ds[KOUT - 1].ap(),
                    in_ap=wout[:].rearrange("p (g e) -> p g e", e=NEMB),
                    idxs_ap=wpos[:],
                    num_idxs=16,
                    num_idxs_reg=16,
                    elem_size=NEMB,
                )

            # projection tiles (K on partitions)
            p1a = sb.tile([P, NEMB], bf, tag="p1a")
            nc.sync.dma_start(p1a[:], proj_d[1].ap()[0:128, :])
            p1b = sb.tile([P, NEMB], bf, tag="p1b")
            nc.sync.dma_start(p1b[:], proj_d[1].ap()[128:256, :])
            p2 = sb.tile([64, NEMB], bf, tag="p2")
            nc.sync.dma_start(p2[:], proj_d[2].ap())
            p3 = sb.tile([16, NEMB], bf, tag="p3")
            nc.sync.dma_start(p3[:], proj_d[3].ap())
            pr_tiles = {1: (p1a, p1b), 2: (p2,), 3: (p3,)}

            alt = 0  # alternate scaled-copy engine between DVE and ACT
            sc = 0   # scatter round-robin counter
            for u in UNITS:
                b, c = u
                cap = caps[u]
                G = cap // P
                w = cap // 16
                idx_ap = meta_t[:, meta_off[u] : meta_off[u] + w]
                pos_ap = meta_t[:, meta_off[u] + w : meta_off[u] + 2 * w]
                if b == 0:
                    g0 = sb.tile([P, G * NEMB], bf, tag=f"g{u}")
                    nc.gpsimd.dma_gather(
                        out_ap=g0[:].rearrange("p (g e) -> p g e", e=NEMB),
                        in_ap=tab_d[u].ap(),
                        idxs_ap=meta_t[:, meta_off[u] : meta_off[u] + caps16[u] // 16],
                        num_idxs=caps16[u],
                        num_idxs_reg=caps16[u],
                        elem_size=NEMB,
                        single_packet=False,
                    )
                    s0 = sb.tile([P, G * NEMB], bf, tag=f"s{u}")
                    nc.vector.tensor_scalar_mul(s0[:], g0[:], SCALE)
                    if DEVICE_SCATTER:
                        nc.gpsimd.dma_scatter_add(
                            out_ap=out_ds[sc % KOUT].ap(),
                            in_ap=s0[:].rearrange("p (g e) -> p g e", e=NEMB),
                            idxs_ap=meta_t[:, meta_off[u] + w : meta_off[u] + w + caps16[u] // 16],
                            num_idxs=caps16[u],
                            num_idxs_reg=caps16[u],
                            elem_size=NEMB,
                            single_packet=False,
                        )
                        sc += 1
                    else:
                        nc.sync.dma_start(
                            outc.ap()[coff[u] : coff[u] + cap, :].rearrange(
                                "(g p) e -> p g e", p=P),
                            s0[:].rearrange("p (g e) -> p g e", e=NEMB),
                        )
                else:
                    d = N_EMBEDS[b]
                    dp = max(d, P)  # padded row width in the bf16 table
                    KC = dp // P    # K chunks of 128
                    et = sb.tile([P, KC * cap], bf, tag=f"g{u}")
                    nc.gpsimd.dma_gather(
                        out_ap=et[:].rearrange("p (k n) -> p k n", n=cap),
                        in_ap=tab_d[u].ap(),
                        idxs_ap=idx_ap,
                        num_idxs=cap,
                        num_idxs_reg=cap,
                        elem_size=dp,
                        transpose=True,
                        single_packet=False,
                    )
                    o = sb.tile([P, G * NEMB], bf, tag=f"s{u}")
                    for g in range(G):
                        pt = ps.tile([P, NEMB], mybir.dt.float32, tag="ps")
                        for n in range(2):
                            if b == 1:
                                nc.tensor.matmul(
                                    out=pt[:, n * 512 : (n + 1) * 512],
                                    lhsT=et[:].rearrange("p (k n) -> p k n", n=cap)[
                                        :, 0, g * P : (g + 1) * P],
                                    rhs=p1a[:, n * 512 : (n + 1) * 512],
                                    start=True, stop=False,
                                )
                                nc.tensor.matmul(
                                    out=pt[:, n * 512 : (n + 1) * 512],
                                    lhsT=et[:].rearrange("p (k n) -> p k n", n=cap)[
                                        :, 1, g * P : (g + 1) * P],
                                    rhs=p1b[:, n * 512 : (n + 1) * 512],
                                    start=False, stop=True,
                                )
                            else:
                                nc.tensor.matmul(
                                    out=pt[:, n * 512 : (n + 1) * 512],
                                    lhsT=et[0:d, g * P : (g + 1) * P],
                                    rhs=pr_tiles[b][0][:, n * 512 : (n + 1) * 512],
                                    start=True, stop=True,
                                )
                        dst = o[:, g * NEMB : (g + 1) * NEMB]
                        if alt % 2 == 0:
                            nc.vector.tensor_scalar_mul(dst, pt[:], SCALE)
                        else:
                            nc.scalar.mul(dst, pt[:], SCALE)
                        alt += 1
                    if DEVICE_SCATTER:
                        nc.gpsimd.dma_scatter_add(
                            out_ap=out_ds[sc % KOUT].ap(),
                            in_ap=o[:].rearrange("p (g e) -> p g e", e=NEMB),
                            idxs_ap=meta_t[:, meta_off[u] + w : meta_off[u] + w + caps16[u] // 16],
                            num_idxs=caps16[u],
                            num_idxs_reg=caps16[u],
                            elem_size=NEMB,
                            single_packet=False,
                        )
                        sc += 1
                    else:
                        nc.sync.dma_start(
                            outc.ap()[coff[u] : coff[u] + cap, :].rearrange(
                                "(g p) e -> p g e", p=P),
                            o[:].rearrange("p (g e) -> p g e", e=NEMB),
                        )
    nc.compile()
    return nc


def _ensure_profile_hook():
    """If BASS_TRACE is set but antenv.axon_hooks is absent (as in this
    container), register a ctypes-based NTFF hook shim so tracing works
    instead of crashing on import."""
    try:
        import antenv.axon_hooks  # noqa: F401
        return
    except ImportError:
        pass
    import contextlib, ctypes, sys, types

    so_path = "/opt/axon/libaxon_pjrt.so"
    hook = None
    try:
        lib = ctypes.CDLL(so_path)
        if hasattr(lib, "axon_start_nrt_profile"):
            lib.axon_start_nrt_profile.argtypes = [
                ctypes.POINTER(ctypes.c_int64), ctypes.c_size_t]
            lib.axon_start_nrt_profile.restype = ctypes.c_int64
            lib.axon_stop_nrt_profile.argtypes = [ctypes.c_char_p]
            lib.axon_stop_nrt_profile.restype = ctypes.c_int64

            @contextlib.contextmanager
            def hook(output_dir, device_ids):
                import jax
                jax.devices()
                if device_ids:
                    ids = (ctypes.c_int64 * len(device_ids))(*device_ids)
                    rc = lib.axon_start_nrt_profile(ids, len(device_ids))
                else:
                    rc = lib.axon_start_nrt_profile(None, 0)
                if rc != 0:
                    raise RuntimeError(f"axon_start_nrt_profile rc={rc}")
                try:
                    yield
                finally:
                    lib.axon_stop_nrt_profile(str(output_dir).encode())
    except OSError:
        pass
    mod = types.ModuleType("antenv.axon_hooks")
    mod.get_axon_ntff_profile_hook = lambda: hook
    mod.set_axon_ntff_profile_hook = lambda h: None
    sys.modules["antenv.axon_hooks"] = mod


def _run(inputs, trace=False):
    _ensure_profile_hook()
    from concourse.bass_utils import run_bass_kernel_spmd

    per, tabs, projs, metas, caps, caps16, meta_off, lists, orders = _prep_host(inputs)
    meta_w = metas[0].shape[1]
    nc = _build(per, tabs, projs, caps, caps16, meta_off, meta_w)

    in_maps = []
    for core in range(N_CORES):
        m = {f"tab{u[0]}_{u[1]}": np.asarray(tabs[u]) for u in UNITS}
        m.update({f"proj{i}": np.asarray(projs[i]) for i in (1, 2, 3)})
        m["meta"] = metas[core]
        in_maps.append(m)
    try:
        res = run_bass_kernel_spmd(
            nc, in_maps, core_ids=list(range(N_CORES)), trace=trace
        )
    except Exception:
        # transient device errors (e.g. NRT exec-unit unrecoverable) usually
        # clear after the terminal watchdog resets the device
        import time as _time

        _time.sleep(90)
        res = run_bass_kernel_spmd(
            nc, in_maps, core_ids=list(range(N_CORES)), trace=trace
        )
    x = np.asarray(inputs["x"])
    ntok = x.size
    full = np.zeros((ntok, NEMB), np.float32)
    if DEVICE_SCATTER:
        for i in range(N_CORES):
            n_i = len(orders[i])
            acc = np.zeros((n_i, NEMB), np.float32)
            for k in range(KOUT):
                acc += np.asarray(res.results[i][f"out{k}"])[:n_i].astype(np.float32)
            full[orders[i]] = acc
    else:
        off = 0
        coff = {}
        for u in UNITS:
            coff[u] = off
            off += caps[u]
        for i in range(N_CORES):
            oc = np.asarray(res.results[i]["outc"])
            for u in UNITS:
                _, gpos = lists[i][u]
                full[gpos] = oc[coff[u] : coff[u] + len(gpos)].astype(np.float32)
    full = full.reshape(*x.shape, NEMB)
    return full, res


def kernel(**inputs) -> np.ndarray:
    out, _ = _run(inputs, trace=False)
    return out



# revision 25
# speedup vs baseline: 1.0607x; 1.0607x over previous
"""AdaptiveEmbedding (adaptive-softmax style embedding lookup) on 8 TRN2
NeuronCores.

Sharding: data-parallel over tokens (4096 tokens/core); bucket tables and
projections are replicated in device DRAM as bf16. Within each core's shard,
tokens are sub-sharded host-side by (bucket, int16-addressable row-range
chunk) -- dma_gather/dma_scatter_add take int16 indices, so each gather unit
covers at most 32768 table rows.

Device program per core (one SPMD NEFF on cores 0-7):
  - bucket 0 (d=1024): dma_gather of hit rows -> DVE scale by sqrt(1024)=32
  - buckets 1-3 (d=256/64/16): transposed dma_gather produces e^T tiles
    ([d on partitions] x [tokens on free dim]) directly; TensorE matmuls
    against the replicated projection (PSUM f32 accumulate); scale-by-32
    PSUM->SBUF copy alternating between DVE and ScalarE.
  - output, two modes:
      DEVICE_SCATTER=True:  dma_scatter_add places each row at its token
        position across KOUT round-robin output tensors (avoids WAW
        serialization between scatter DMAs); host sums the disjoint partials.
      DEVICE_SCATTER=False (default, ~40us faster): each unit's rows are
        written contiguously (plain HWDGE DMA); the host unshard step places
        each sub-shard's rows at their token positions. This removes the
        per-row scatter descriptor generation, which is GpSimd-bound
        (~10ns/row on the Q7 SWDGE ucode) and is the kernel's bottleneck.

Output is bf16 on device (rel err ~3e-3), upcast to f32 on host.
"""
import math
import numpy as np
import ml_dtypes

N_VOCAB = 267735
STARTS = [0, 20000, 40000, 200000]
ENDS = [20000, 40000, 200000, N_VOCAB]
N_EMBEDS = [1024, 256, 64, 16]
N_CORES = 8
NEMB = 1024
SCALE = 32.0  # sqrt(1024)
KOUT = 4   # independent scatter target tensors
DEVICE_SCATTER = False  # False: device writes bucket-contiguous, host places rows
CHUNK = 32768  # int16-addressable rows per gather chunk
P = 128

# static unit list: (bucket, chunk_index)
UNITS = []
for _b in range(4):
    _nr = ENDS[_b] - STARTS[_b]
    for _c in range(math.ceil(_nr / CHUNK)):
        UNITS.append((_b, _c))


def _wrap16(a):
    # [N] -> [16, N/16] wrapped, replicated to 128 partitions
    w = a.reshape(-1, 16).T.astype(np.int16)
    return np.tile(w, (8, 1))


def _prep_host(inputs):
    x = np.asarray(inputs["x"]).astype(np.int64).reshape(-1)
    ntok = x.size
    assert ntok % N_CORES == 0, f"token count {ntok} not divisible by {N_CORES}"
    per = ntok // N_CORES

    bf = ml_dtypes.bfloat16
    tabs = {}
    for b in range(4):
        t = np.asarray(inputs[f"table{b}"], np.float32)
        d = N_EMBEDS[b]
        if d < P:  # pad rows to 128 elements (256B in bf16) for dma_gather
            tp = np.zeros((t.shape[0], P), np.float32)
            tp[:, :d] = t
            t = tp
        tb = t.astype(bf)
        nr = t.shape[0]
        for c in range(math.ceil(nr / CHUNK)):
            tabs[(b, c)] = np.ascontiguousarray(tb[c * CHUNK : (c + 1) * CHUNK])
    projs = {
        i: np.asarray(inputs[f"proj{i}"], np.float32).astype(bf) for i in (1, 2, 3)
    }

    # per-core, per-unit token lists
    counts = {u: [] for u in UNITS}
    lists = []
    for core in range(N_CORES):
        xs = x[core * per : (core + 1) * per]
        d = {}
        for (b, c) in UNITS:
            lo = STARTS[b] + c * CHUNK
            hi = min(STARTS[b] + (c + 1) * CHUNK, ENDS[b])
            pos = np.nonzero((xs >= lo) & (xs < hi))[0]
            lid = xs[pos] - lo
            d[(b, c)] = (lid, pos)
            counts[(b, c)].append(len(pos))
        lists.append(d)

    caps = {
        u: max(P, -(-max(counts[u]) // P) * P) for u in UNITS
    }  # multiple of 128, >= max count (gather/transposed-gather size)
    caps16 = {
        u: min(caps[u], max(16, -(-max(counts[u]) // 16) * 16)) for u in UNITS
    }  # multiple of 16 -- used for scatters and the b0 gather

    # meta tensor: per unit [idx_wrapped | pos_wrapped], concat along free dim
    metas = []
    meta_off = {}
    off = 0
    for u in UNITS:
        w = caps[u] // 16
        meta_off[u] = off
        off += 2 * w
    for core in range(N_CORES):
        cols = []
        for u in UNITS:
            lid, pos = lists[core][u]
            cap = caps[u]
            # pad with valid entries so every core has exactly `cap` live
            # indices: idx 0 (gathers row 0, discarded) and dummy output rows
            # [per, per+128) (sliced off on the host).
            il = np.zeros(cap, np.int64)
            pl = per + (np.arange(cap, dtype=np.int64) % P)
            il[: len(lid)] = lid
            pl[: len(pos)] = pos
            cols.append(_wrap16(il))
            cols.append(_wrap16(pl))
        metas.append(np.concatenate(cols, axis=1))
    return per, tabs, projs, metas, caps, caps16, meta_off, lists


def _build(per, tabs, projs, caps, caps16, meta_off, meta_w):
    import concourse.bass as bass
    import concourse.tile as tile
    from concourse import bacc, mybir

    bf = mybir.dt.bfloat16
    nc = bacc.Bacc("TRN2", target_bir_lowering=False, debug=False)

    tab_d = {
        u: nc.dram_tensor(f"tab{u[0]}_{u[1]}", list(tabs[u].shape), bf,
                          kind="ExternalInput")
        for u in UNITS
    }
    proj_d = {
        i: nc.dram_tensor(f"proj{i}", list(projs[i].shape), bf,
                          kind="ExternalInput")
        for i in (1, 2, 3)
    }
    meta_d = nc.dram_tensor("meta", [P, meta_w], mybir.dt.int16,
                            kind="ExternalInput")
    # KOUT independent output tensors (scatters round-robin over them) so
    # consecutive scatter DMAs have no WAW edge and overlap; rows are
    # disjoint across units, host sums the partials.
    if DEVICE_SCATTER:
        out_ds = [
            nc.dram_tensor(f"out{k}", [per + P, NEMB], bf, kind="ExternalOutput")
            for k in range(KOUT)
        ]
        outc = None
    else:
        ncap = sum(caps.values())
        outc = nc.dram_tensor("outc", [ncap, NEMB], bf, kind="ExternalOutput")
        out_ds = [outc]  # unused in this mode (warmup scatter is skipped)
        coff = {}
        off = 0
        for u in UNITS:
            coff[u] = off
            off += caps[u]

    with tile.TileContext(nc) as tc:
        with (
            tc.tile_pool(name="sb", bufs=1) as sb,
            tc.tile_pool(name="ps", bufs=4, space="PSUM") as ps,
        ):
            meta_t = sb.tile([P, meta_w], mybir.dt.int16, tag="meta")
            nc.gpsimd.dma_start(meta_t[:], meta_d.ap())

            # Warmup: tiny gather (+scatter in scatter mode) with no data
            # deps, so the one-time custom-op ucode load overlaps the
            # pipeline head.
            widx = sb.tile([P, 1], mybir.dt.int16, tag="widx")
            nc.gpsimd.memset(widx[:], 0)
            wpos = sb.tile([P, 1], mybir.dt.int16, tag="wpos")
            nc.gpsimd.memset(wpos[:], per)  # dummy output row
            wout = sb.tile([P, NEMB], bf, tag="wout")
            nc.gpsimd.dma_gather(
                out_ap=wout[:].rearrange("p (g e) -> p g e", e=NEMB),
                in_ap=proj_d[3].ap(),
                idxs_ap=widx[:],
                num_idxs=16,
                num_idxs_reg=16,
                elem_size=NEMB,
            )
            if DEVICE_SCATTER:
                nc.gpsimd.dma_scatter_add(
                    out_ap=out_ds[KOUT - 1].ap(),
                    in_ap=wout[:].rearrange("p (g e) -> p g e", e=NEMB),
                    idxs_ap=wpos[:],
                    num_idxs=16,
                    num_idxs_reg=16,
                    elem_size=NEMB,
                )

            # projection tiles (K on partitions)
            p1a = sb.tile([P, NEMB], bf, tag="p1a")
            nc.sync.dma_start(p1a[:], proj_d[1].ap()[0:128, :])
            p1b = sb.tile([P, NEMB], bf, tag="p1b")
            nc.sync.dma_start(p1b[:], proj_d[1].ap()[128:256, :])
            p2 = sb.tile([64, NEMB], bf, tag="p2")
            nc.sync.dma_start(p2[:], proj_d[2].ap())
            p3 = sb.tile([16, NEMB], bf, tag="p3")
            nc.sync.dma_start(p3[:], proj_d[3].ap())
            pr_tiles = {1: (p1a, p1b), 2: (p2,), 3: (p3,)}

            alt = 0  # alternate scaled-copy engine between DVE and ACT
            sc = 0   # scatter round-robin counter
            for u in UNITS:
                b, c = u
                cap = caps[u]
                G = cap // P
                w = cap // 16
                idx_ap = meta_t[:, meta_off[u] : meta_off[u] + w]
                pos_ap = meta_t[:, meta_off[u] + w : meta_off[u] + 2 * w]
                if b == 0:
                    g0 = sb.tile([P, G * NEMB], bf, tag=f"g{u}")
                    nc.gpsimd.dma_gather(
                        out_ap=g0[:].rearrange("p (g e) -> p g e", e=NEMB),
                        in_ap=tab_d[u].ap(),
                        idxs_ap=meta_t[:, meta_off[u] : meta_off[u] + caps16[u] // 16],
                        num_idxs=caps16[u],
                        num_idxs_reg=caps16[u],
                        elem_size=NEMB,
                        single_packet=False,
                    )
                    s0 = sb.tile([P, G * NEMB], bf, tag=f"s{u}")
                    nc.vector.tensor_scalar_mul(s0[:], g0[:], SCALE)
                    if DEVICE_SCATTER:
                        nc.gpsimd.dma_scatter_add(
                            out_ap=out_ds[sc % KOUT].ap(),
                            in_ap=s0[:].rearrange("p (g e) -> p g e", e=NEMB),
                            idxs_ap=meta_t[:, meta_off[u] + w : meta_off[u] + w + caps16[u] // 16],
                            num_idxs=caps16[u],
                            num_idxs_reg=caps16[u],
                            elem_size=NEMB,
                            single_packet=False,
                        )
                        sc += 1
                    else:
                        nc.sync.dma_start(
                            outc.ap()[coff[u] : coff[u] + cap, :].rearrange(
                                "(g p) e -> p g e", p=P),
                            s0[:].rearrange("p (g e) -> p g e", e=NEMB),
                        )
                else:
                    d = N_EMBEDS[b]
                    dp = max(d, P)  # padded row width in the bf16 table
                    KC = dp // P    # K chunks of 128
                    et = sb.tile([P, KC * cap], bf, tag=f"g{u}")
                    nc.gpsimd.dma_gather(
                        out_ap=et[:].rearrange("p (k n) -> p k n", n=cap),
                        in_ap=tab_d[u].ap(),
                        idxs_ap=idx_ap,
                        num_idxs=cap,
                        num_idxs_reg=cap,
                        elem_size=dp,
                        transpose=True,
                        single_packet=False,
                    )
                    o = sb.tile([P, G * NEMB], bf, tag=f"s{u}")
                    for g in range(G):
                        pt = ps.tile([P, NEMB], mybir.dt.float32, tag="ps")
                        for n in range(2):
                            if b == 1:
                                nc.tensor.matmul(
                                    out=pt[:, n * 512 : (n + 1) * 512],
                                    lhsT=et[:].rearrange("p (k n) -> p k n", n=cap)[
                                        :, 0, g * P : (g + 1) * P],
                                    rhs=p1a[:, n * 512 : (n + 1) * 512],
                                    start=True, stop=False,
                                )
                                nc.tensor.matmul(
                                    out=pt[:, n * 512 : (n + 1) * 512],
                                    lhsT=et[:].rearrange("p (k n) -> p k n", n=cap)[
                                        :, 1, g * P : (g + 1) * P],
                                    rhs=p1b[:, n * 512 : (n + 1) * 512],
                                    start=False, stop=True,
                                )
                            else:
                                nc.tensor.matmul(
                                    out=pt[:, n * 512 : (n + 1) * 512],
                                    lhsT=et[0:d, g * P : (g + 1) * P],
                                    rhs=pr_tiles[b][0][:, n * 512 : (n + 1) * 512],
                                    start=True, stop=True,
                                )
                        dst = o[:, g * NEMB : (g + 1) * NEMB]
                        if alt % 2 == 0:
                            nc.vector.tensor_scalar_mul(dst, pt[:], SCALE)
                        else:
                            nc.scalar.mul(dst, pt[:], SCALE)
                        alt += 1
                    if DEVICE_SCATTER:
                        nc.gpsimd.dma_scatter_add(
                            out_ap=out_ds[sc % KOUT].ap(),
                            in_ap=o[:].rearrange("p (g e) -> p g e", e=NEMB),
                            idxs_ap=meta_t[:, meta_off[u] + w : meta_off[u] + w + caps16[u] // 16],
                            num_idxs=caps16[u],
                            num_idxs_reg=caps16[u],
                            elem_size=NEMB,
                            single_packet=False,
                        )
                        sc += 1
                    else:
                        nc.sync.dma_start(
                            outc.ap()[coff[u] : coff[u] + cap, :].rearrange(
                                "(g p) e -> p g e", p=P),
                            o[:].rearrange("p (g e) -> p g e", e=NEMB),
                        )
    nc.compile()
    return nc


def _ensure_profile_hook():
    """If BASS_TRACE is set but antenv.axon_hooks is absent (as in this
    container), register a ctypes-based NTFF hook shim so tracing works
    instead of crashing on import."""
    try:
        import antenv.axon_hooks  # noqa: F401
        return
    except ImportError:
        pass
    import contextlib, ctypes, sys, types

    so_path = "/opt/axon/libaxon_pjrt.so"
    hook = None
    try:
        lib = ctypes.CDLL(so_path)
        if hasattr(lib, "axon_start_nrt_profile"):
            lib.axon_start_nrt_profile.argtypes = [
                ctypes.POINTER(ctypes.c_int64), ctypes.c_size_t]
            lib.axon_start_nrt_profile.restype = ctypes.c_int64
            lib.axon_stop_nrt_profile.argtypes = [ctypes.c_char_p]
            lib.axon_stop_nrt_profile.restype = ctypes.c_int64

            @contextlib.contextmanager
            def hook(output_dir, device_ids):
                import jax
                jax.devices()
                if device_ids:
                    ids = (ctypes.c_int64 * len(device_ids))(*device_ids)
                    rc = lib.axon_start_nrt_profile(ids, len(device_ids))
                else:
                    rc = lib.axon_start_nrt_profile(None, 0)
                if rc != 0:
                    raise RuntimeError(f"axon_start_nrt_profile rc={rc}")
                try:
                    yield
                finally:
                    lib.axon_stop_nrt_profile(str(output_dir).encode())
    except OSError:
        pass
    mod = types.ModuleType("antenv.axon_hooks")
    mod.get_axon_ntff_profile_hook = lambda: hook
    mod.set_axon_ntff_profile_hook = lambda h: None
    sys.modules["antenv.axon_hooks"] = mod


def _run(inputs, trace=False):
    _ensure_profile_hook()
    from concourse.bass_utils import run_bass_kernel_spmd

    per, tabs, projs, metas, caps, caps16, meta_off, lists = _prep_host(inputs)
    meta_w = metas[0].shape[1]
    nc = _build(per, tabs, projs, caps, caps16, meta_off, meta_w)

    in_maps = []
    for core in range(N_CORES):
        m = {f"tab{u[0]}_{u[1]}": np.asarray(tabs[u]) for u in UNITS}
        m.update({f"proj{i}": np.asarray(projs[i]) for i in (1, 2, 3)})
        m["meta"] = metas[core]
        in_maps.append(m)
    try:
        res = run_bass_kernel_spmd(
            nc, in_maps, core_ids=list(range(N_CORES)), trace=trace
        )
    except Exception:
        # transient device errors (e.g. NRT exec-unit unrecoverable) usually
        # clear after the terminal watchdog resets the device
        import time as _time

        _time.sleep(90)
        res = run_bass_kernel_spmd(
            nc, in_maps, core_ids=list(range(N_CORES)), trace=trace
        )
    x = np.asarray(inputs["x"])
    outs = []
    if DEVICE_SCATTER:
        for i in range(N_CORES):
            acc = np.zeros((per, NEMB), np.float32)
            for k in range(KOUT):
                acc += np.asarray(res.results[i][f"out{k}"])[:per].astype(np.float32)
            outs.append(acc)
    else:
        off = 0
        coff = {}
        for u in UNITS:
            coff[u] = off
            off += caps[u]
        for i in range(N_CORES):
            oc = np.asarray(res.results[i]["outc"])
            acc = np.zeros((per, NEMB), np.float32)
            for u in UNITS:
                _, pos = lists[i][u]
                acc[pos] = oc[coff[u] : coff[u] + len(pos)].astype(np.float32)
            outs.append(acc)
    full = np.concatenate(outs, axis=0).reshape(*x.shape, NEMB)
    return full, res


def kernel(**inputs) -> np.ndarray:
    out, _ = _run(inputs, trace=False)
    return out

